# revision 24
# baseline (speedup 1.0000x reference)
"""Trainium2 Bass kernel: 8-layer ternary (BitNet-1.58) dense transformer.

Model (per reference):
    h = embed[input_ids]                                  # (B=2, S=1024, H=2048)
    8x: y = h @ ternary(W_l)^T + b_l ; h = LN(y + h)*g+b  # H=2048
    h = LN(h)*final_g + final_b
    logits = h @ ternary(head_W)^T                        # (B, S, V=32000)

Sharding over 8 NeuronCores (fully local, no collectives):
  - Layers: data-parallel over the 2048 tokens (256 tokens/core). Each core
    streams the full ternary layer weights as exact {-1,0,+1} fp8(e4m3).
  - Head: ALSO data-parallel over tokens: each core computes its own 256
    tokens x the full 32000-entry vocab, streaming fp8 head weights
    chunk-by-chunk, overlapped with compute. No collectives at all.

Head matmul runs mixed precision: k-tiles 0..5 via fp8 DoubleRow (2 k-tiles
per instruction, activations rounded to e4m3), k-tiles 6..15 as bf16
activations x fp8 weights at full precision. The e4m3 rounding of 6/16 of
the contraction costs ~1.6e-2 relative error on the logits (vs the 2e-2
budget) and saves ~16% of the dominant matmul stream time.

When the LN affine params and biases are identity (they are for this model
instance; checked at runtime with a general fallback), the final LayerNorm
is also skipped: its input is already a LayerNorm output (per-token mean
exactly 0, variance 1-eps/var), so the final LN is an identity up to
O(eps)~2.5e-6.

HW notes (found the hard way): a ScalarE read of a full 2048B PSUM bank
hard-faults the exec unit -- all ScalarE PSUM reads here are <=1536B.
Activation transposes run as bf16 (2x faster through the PE than f32),
with a bf16 identity matrix as the moving operand.
"""

import os
import sys

import numpy as np

try:
    import concourse.bass as bass
except ImportError:  # grading container should have it on sys.path already
    sys.path.insert(0, "/opt/trn_rl_repo")
    import concourse.bass as bass

import ml_dtypes
import concourse.mybir as mybir
import concourse.tile as tile
from concourse import bacc
from concourse.bass_utils import run_bass_kernel_spmd
from contextlib import ExitStack

F32 = mybir.dt.float32
BF16 = mybir.dt.bfloat16
FP8E4 = mybir.dt.float8e4
AX = mybir.AxisListType
OP = mybir.AluOpType
AF = mybir.ActivationFunctionType
DR = mybir.MatmulPerfMode.DoubleRow
EPS = 1e-5

# Full-size problem config (B=2, S=1024 -> 2048 tokens, 256/core).
# Head: vocab padded 32000 -> 63*512; k-tiles 0..DRKT-1 run as fp8 DoubleRow.
CFG_FULL = dict(L=8, H=2048, NC=8, TT=2, V=32000, QV=512, NQ=63, CH=512, DRKT=6)


def build_nc(cfg, scales, head_scale, triv_ln, fp8_w, use_dr):
    L, H, NC, TT = cfg["L"], cfg["H"], cfg["NC"], cfg["TT"]
    V, QV, NQ, CH, DRKT = cfg["V"], cfg["QV"], cfg["NQ"], cfg["CH"], cfg["DRKT"]
    KT = H // 128
    KQ = KT // 4  # k-tiles per layer-weight quarter
    NCH = H // CH
    DRP = DRKT // 2
    if not use_dr:
        DRKT = DRP = 0
    assert H % CH == 0 and NQ * QV >= V
    WDT = FP8E4 if fp8_w else BF16

    nc = bacc.Bacc("TRN2", target_bir_lowering=False, debug=False, num_devices=NC)
    h0 = nc.declare_dram_parameter("h0", [TT, 128, H], F32, isOutput=False)
    h0T = nc.declare_dram_parameter("h0T", [TT, 128, H], BF16, isOutput=False)
    # weights pre-arranged on host: [L, 128part, KT, H] -> contiguous
    # 8KB-per-partition quarter loads (fast DMA descriptor issue)
    w_ = nc.declare_dram_parameter("w", [L, 128, KT, H], WDT, isOutput=False)
    if not triv_ln:
        lng = nc.declare_dram_parameter("lng", [L, H], BF16, isOutput=False)
        lnb = nc.declare_dram_parameter("lnb", [L, H], BF16, isOutput=False)
        lbias = nc.declare_dram_parameter("lbias", [L, H], BF16, isOutput=False)
        fing = nc.declare_dram_parameter("fing", [H], BF16, isOutput=False)
        finb = nc.declare_dram_parameter("finb", [H], BF16, isOutput=False)
    hw_ = nc.declare_dram_parameter("hw", [NQ, 128, KT, QV], WDT, isOutput=False)
    identb_d = nc.declare_dram_parameter("identb", [128, 128], BF16, isOutput=False)
    eps_d = nc.declare_dram_parameter("eps", [128, 1], F32, isOutput=False)
    out = nc.declare_dram_parameter("out", [TT * 128, V], F32, isOutput=True)

    with tile.TileContext(nc) as tc:
        with ExitStack() as ctx0:
            consts = ctx0.enter_context(tc.tile_pool(name="consts", bufs=1))
            state = ctx0.enter_context(tc.tile_pool(name="state", bufs=4))
            hTp = ctx0.enter_context(tc.tile_pool(name="hT", bufs=2))
            hT8p = ctx0.enter_context(tc.tile_pool(name="hT8", bufs=2))
            wqp = ctx0.enter_context(tc.tile_pool(name="wq", bufs=4))
            outp = ctx0.enter_context(tc.tile_pool(name="outstg", bufs=6))
            smp = ctx0.enter_context(tc.tile_pool(name="small", bufs=16))

            identb = consts.tile([128, 128], BF16, name="identb")
            nc.sync.dma_start(identb[:], identb_d[:])
            eps_t = consts.tile([128, 1], F32, name="epst")
            nc.sync.dma_start(eps_t[:], eps_d[:])

            h_cur = []
            hT_cur = []
            for t in range(TT):
                hTt = hTp.tile([128, H], BF16, tag="hT", name=f"hT_p{t}")
                hT_cur.append(hTt)
                st = state.tile([128, H], F32, name=f"hinit{t}", tag="state")
                h_cur.append(st)
            hT8_cur = [None] * TT

            with ExitStack() as ctxA:
                zpool = ctxA.enter_context(tc.tile_pool(name="z", bufs=2))
                wp = ctxA.enter_context(tc.tile_pool(name="w", bufs=12))
                sqp = ctxA.enter_context(tc.tile_pool(name="sq", bufs=2))
                hSp = ctxA.enter_context(tc.tile_pool(name="hS", bufs=2))
                gbp = None
                if not triv_ln:
                    gbp = ctxA.enter_context(tc.tile_pool(name="gb", bufs=2))
                psT = ctxA.enter_context(
                    tc.tile_pool(name="psT", bufs=2, space="PSUM")
                )
                psY = ctxA.enter_context(
                    tc.tile_pool(name="psY", bufs=NCH + 2, space="PSUM")
                )

                def load_wq(l, nsplit=8):
                    w_q = []
                    kq = KT // nsplit
                    for hf in range(nsplit):
                        wt = wp.tile([128, kq, H], WDT, tag="w", name=f"w{l}_{hf}")
                        nc.sync.dma_start(
                            wt[:], w_[l, :, hf * kq : (hf + 1) * kq, :]
                        )
                        w_q.append(wt)
                    return w_q, kq

                # first lhsT tile, then layer-0 weights in small slices so
                # the very first matmuls unblock with minimal DMA in flight
                nc.sync.dma_start(hT_cur[0][:, : H // 2], h0T[0][:, : H // 2])
                w_pre = load_wq(0, nsplit=8)
                nc.sync.dma_start(hT_cur[0][:, H // 2 :], h0T[0][:, H // 2 :])
                nc.sync.dma_start(hT_cur[1][:], h0T[1])
                for t in range(TT):
                    nc.sync.dma_start(h_cur[t][:], h0[t])

                def transpose_bf(src_bf, name):
                    """bf16 pre-scaled [128tok, H] -> hT bf16 [128feat-in-
                    block, (kt,128tok)]; 1-bank bf16 psum halves, ScalarE
                    psum reads kept at 1024B (full-bank reads fault)."""
                    dst = hTp.tile([128, H], BF16, tag="hT", name=f"hT{name}")
                    for hf in range(2):
                        pT = psT.tile(
                            [128, H // 2], BF16, tag="psTb", name=f"pT{name}_{hf}"
                        )
                        for k in range(KT // 2):
                            kt = hf * (KT // 2) + k
                            nc.tensor.transpose(
                                pT[:, k * 128 : (k + 1) * 128],
                                src_bf[:, kt * 128 : (kt + 1) * 128],
                                identb[:],
                            )
                        lo = hf * (H // 2)
                        nc.scalar.copy(dst[:, lo : lo + 512], pT[:, :512])
                        nc.scalar.copy(dst[:, lo + 512 : lo + 1024], pT[:, 512:])
                    return dst

                def transpose_head(src_scaled_bf, src_unscaled_bf, name):
                    """Final (head) lhsT: kt 0..DRKT-1 transposed from the
                    UNSCALED bf16 activations -> e4m3 (DoubleRow operand);
                    kt DRKT..15 from the head_scale-folded bf16 ones."""
                    dst = hTp.tile([128, H], BF16, tag="hT", name=f"hT{name}")
                    dst8 = None
                    if DRKT:
                        dst8 = hT8p.tile(
                            [128, DRKT, 128], FP8E4, tag="hT8", name=f"hT8{name}"
                        )
                        pX = psT.tile(
                            [128, H // 2], BF16, tag="psTb", name=f"pX{name}"
                        )
                        for kt in range(DRKT):
                            nc.tensor.transpose(
                                pX[:, kt * 128 : (kt + 1) * 128],
                                src_unscaled_bf[:, kt * 128 : (kt + 1) * 128],
                                identb[:],
                            )
                        nc.scalar.copy(
                            dst8[:].rearrange("p a b -> p (a b)"),
                            pX[:, : DRKT * 128],
                        )
                    rem = list(range(DRKT, KT))
                    for c0 in range(0, len(rem), KT // 2):
                        grp = rem[c0 : c0 + KT // 2]
                        pT = psT.tile(
                            [128, H // 2], BF16, tag="psTb", name=f"pY{name}_{c0}"
                        )
                        for j, kt in enumerate(grp):
                            nc.tensor.transpose(
                                pT[:, j * 128 : (j + 1) * 128],
                                src_scaled_bf[:, kt * 128 : (kt + 1) * 128],
                                identb[:],
                            )
                        n = len(grp) * 128
                        lo = grp[0] * 128
                        half = min(512, n)
                        nc.scalar.copy(dst[:, lo : lo + half], pT[:, :half])
                        if half < n:
                            nc.scalar.copy(
                                dst[:, lo + half : lo + n], pT[:, half:n]
                            )
                    return dst, dst8

                def ln_core(z, sums, name):
                    """Shared LN statistics tail: returns (negmean, rstd)."""
                    S = smp.tile([128, 1], F32, tag="s0", name=f"S{name}")
                    SS = smp.tile([128, 1], F32, tag="s1", name=f"SS{name}")
                    nc.vector.tensor_reduce(
                        S[:], sums[:, 0:1], axis=AX.X, op=OP.add
                    )
                    nc.vector.tensor_reduce(
                        SS[:], sums[:, NCH : 2 * NCH], axis=AX.X, op=OP.add
                    )
                    negmean = smp.tile([128, 1], F32, tag="s2", name=f"nm{name}")
                    nc.vector.tensor_scalar_mul(negmean[:], S[:], -1.0 / H)
                    msq = smp.tile([128, 1], F32, tag="s3", name=f"msq{name}")
                    nc.vector.tensor_scalar_mul(msq[:], SS[:], 1.0 / H)
                    var = smp.tile([128, 1], F32, tag="s4", name=f"var{name}")
                    nc.vector.tensor_tensor(var[:], negmean[:], negmean[:], OP.mult)
                    nc.vector.tensor_tensor(var[:], msq[:], var[:], OP.subtract)
                    std = smp.tile([128, 1], F32, tag="s5", name=f"std{name}")
                    nc.scalar.activation(std[:], var[:], AF.Sqrt, bias=eps_t[:])
                    rstd = smp.tile([128, 1], F32, tag="s6", name=f"rstd{name}")
                    nc.vector.reciprocal(rstd[:], std[:])
                    return negmean, rstd

                fg = fb = None
                if not triv_ln:
                    fg = gbp.tile([128, H], BF16, tag="g", name="gfin")
                    nc.sync.dma_start(fg[:], fing[None, :].to_broadcast((128, H)))
                    fb = gbp.tile([128, H], BF16, tag="b", name="bfin")
                    nc.sync.dma_start(fb[:], finb[None, :].to_broadcast((128, H)))

                for l in range(L):
                    w_q, kq = w_pre if l == 0 else load_wq(l)
                    g_t = b_t = bias_t = None
                    if not triv_ln:
                        g_t = gbp.tile([128, H], BF16, tag="g", name=f"g{l}")
                        nc.sync.dma_start(
                            g_t[:], lng[l][None, :].to_broadcast((128, H))
                        )
                        b_t = gbp.tile([128, H], BF16, tag="b", name=f"b{l}")
                        nc.sync.dma_start(
                            b_t[:], lnb[l][None, :].to_broadcast((128, H))
                        )
                        bias_t = gbp.tile(
                            [128, H], BF16, tag="bias", name=f"bias{l}"
                        )
                        nc.sync.dma_start(
                            bias_t[:], lbias[l][None, :].to_broadcast((128, H))
                        )

                    last = l == L - 1
                    next_scale = scales[l + 1] if not last else head_scale
                    for t in range(TT):
                        hTt = hT_cur[t]
                        ps = [
                            psY.tile([128, CH], F32, tag="psY", name=f"ps{l}_{t}_{i}")
                            for i in range(NCH)
                        ]
                        for half in range(2):
                            for kt in range(KT):
                                wt = w_q[kt // kq]
                                for i in (2 * half, 2 * half + 1):
                                    nc.tensor.matmul(
                                        ps[i][:],
                                        lhsT=hTt[:, kt * 128 : (kt + 1) * 128],
                                        rhs=wt[:, kt % kq, i * CH : (i + 1) * CH],
                                        start=(kt == 0),
                                        stop=(kt == KT - 1),
                                    )
                        z = zpool.tile([128, H], F32, tag="z", name=f"z{l}_{t}")
                        sums = smp.tile(
                            [128, 2 * NCH], F32, tag="sums", name=f"sm{l}_{t}"
                        )
                        resid = h_cur[t]
                        if not triv_ln:
                            hb = zpool.tile(
                                [128, H], F32, tag="hb", name=f"hb{l}_{t}"
                            )
                            nc.vector.tensor_tensor(
                                hb[:], h_cur[t][:], bias_t[:], OP.add
                            )
                            resid = hb
                        for i in range(NCH):
                            nc.vector.tensor_add(
                                z[:, i * CH : (i + 1) * CH],
                                ps[i][:],
                                resid[:, i * CH : (i + 1) * CH],
                            )
                        nc.vector.tensor_reduce(
                            sums[:, 0:1], z[:], axis=AX.X, op=OP.add
                        )
                        for i in range(NCH):
                            sq = sqp.tile(
                                [128, CH], BF16, tag="sq", name=f"sq{l}_{t}_{i}"
                            )
                            nc.scalar.activation(
                                sq[:],
                                z[:, i * CH : (i + 1) * CH],
                                AF.Square,
                                accum_out=sums[:, NCH + i : NCH + i + 1],
                            )
                        negmean, rstd = ln_core(z, sums, f"{l}_{t}")
                        rstd_s = smp.tile(
                            [128, 1], F32, tag="s7", name=f"rs{l}_{t}"
                        )
                        nc.vector.tensor_scalar_mul(
                            rstd_s[:], rstd[:], float(next_scale)
                        )

                        if triv_ln and last:
                            # hn is mean-0/var-1 by construction, so the
                            # final identity-affine LN is a no-op to O(eps):
                            # emit one UNSCALED bf16 activation feeding both
                            # head operand sets -- no f32 state, no stats.
                            hnS = hSp.tile(
                                [128, H], BF16, tag="hS", name=f"hS{l}_{t}"
                            )
                            nc.vector.tensor_scalar(
                                hnS[:], z[:], negmean[:], rstd[:],
                                OP.add, OP.mult,
                            )
                            hT_cur[t], hT8_cur[t] = transpose_head(
                                hnS, hnS, f"fin{t}"
                            )
                            continue

                        hn = state.tile([128, H], F32, tag="state", name=f"h{l}_{t}")
                        nc.vector.tensor_scalar(
                            hn[:], z[:], negmean[:], rstd[:], OP.add, OP.mult
                        )
                        if not triv_ln:
                            nc.vector.tensor_tensor(hn[:], hn[:], g_t[:], OP.mult)
                            nc.vector.tensor_tensor(hn[:], hn[:], b_t[:], OP.add)
                        h_cur[t] = hn
                        if not last:
                            hnS = hSp.tile(
                                [128, H], BF16, tag="hS", name=f"hS{l}_{t}"
                            )
                            if triv_ln:
                                nc.vector.tensor_scalar(
                                    hnS[:], z[:], negmean[:], rstd_s[:],
                                    OP.add, OP.mult,
                                )
                            else:
                                nc.vector.tensor_scalar_mul(
                                    hnS[:], hn[:], float(next_scale)
                                )
                            hT_cur[t] = transpose_bf(hnS, f"{l}_{t}")

                if not triv_ln:
                    # general path: true final LN, f32 transposes
                    for t in range(TT):
                        h8 = h_cur[t]
                        sums = smp.tile(
                            [128, 2 * NCH], F32, tag="sums", name=f"smf{t}"
                        )
                        nc.vector.tensor_reduce(
                            sums[:, 0:1], h8[:], axis=AX.X, op=OP.add
                        )
                        for i in range(NCH):
                            sq = sqp.tile(
                                [128, CH], BF16, tag="sq", name=f"sqf{t}_{i}"
                            )
                            nc.scalar.activation(
                                sq[:],
                                h8[:, i * CH : (i + 1) * CH],
                                AF.Square,
                                accum_out=sums[:, NCH + i : NCH + i + 1],
                            )
                        negmean, rstd = ln_core(h8, sums, f"fin{t}")
                        hfin = state.tile(
                            [128, H], F32, tag="state", name=f"hf{t}"
                        )
                        nc.vector.tensor_scalar(
                            hfin[:], h8[:], negmean[:], rstd[:], OP.add, OP.mult
                        )
                        nc.vector.tensor_tensor(hfin[:], hfin[:], fg[:], OP.mult)
                        nc.vector.tensor_tensor(hfin[:], hfin[:], fb[:], OP.add)
                        hnS = hSp.tile([128, H], BF16, tag="hS", name=f"hSf{t}")
                        nc.vector.tensor_scalar_mul(hnS[:], hfin[:], 1.0)
                        hT_cur[t], hT8_cur[t] = transpose_head(
                            hnS, hnS, f"fin{t}"
                        )

            # ---- head: own 256 tokens x full vocab, streamed fp8 weights ----
            with ExitStack() as ctxB:
                psH = ctxB.enter_context(
                    tc.tile_pool(name="psH", bufs=8, space="PSUM")
                )

                def head_group(q, t, wq):
                    # single accumulation group: DR (e4m3) k-tiles then bf16
                    # k-tiles into one bank; both lhsT operands are UNSCALED,
                    # the ternary head scale is applied once at drain.
                    cols = min(QV, V - q * QV)
                    pb = psH.tile([128, QV], F32, tag="psH", name=f"pb{q}_{t}")
                    for j in range(DRP):
                        nc.tensor.matmul(
                            pb[:],
                            lhsT=hT8_cur[t][:, 2 * j : 2 * j + 2, :],
                            rhs=wq[:, 2 * j : 2 * j + 2, :],
                            start=(j == 0),
                            stop=False,
                            perf_mode=DR,
                            skip_group_check=True,
                        )
                    for kt in range(DRKT, KT):
                        nc.tensor.matmul(
                            pb[:],
                            lhsT=hT_cur[t][:, kt * 128 : (kt + 1) * 128],
                            rhs=wq[:, kt, :],
                            start=(DRP == 0 and kt == DRKT),
                            stop=(kt == KT - 1),
                            skip_group_check=True,
                        )
                    # ScalarE PSUM reads must stay under one full 2048B bank
                    hqv = QV // 2
                    o_t = outp.tile([128, QV], F32, tag="ostg", name=f"o{q}_{t}")
                    nc.scalar.activation(
                        o_t[:, :hqv], pb[:, :hqv], AF.Copy,
                        scale=float(head_scale),
                    )
                    nc.scalar.activation(
                        o_t[:, hqv:], pb[:, hqv:], AF.Copy,
                        scale=float(head_scale),
                    )
                    nc.sync.dma_start(
                        out[t * 128 : (t + 1) * 128, q * QV : q * QV + cols],
                        o_t[:, :cols],
                    )

                # first chunks on tile 0 only while tile 1's final
                # transposes land; their tile-1 groups run at the end
                WARM = 2
                for q in range(NQ):
                    wq = wqp.tile([128, KT, QV], WDT, tag="wq", name=f"wq{q}")
                    nc.sync.dma_start(wq[:], hw_[q])
                    for t in [0] if q < WARM else range(TT):
                        head_group(q, t, wq)
                for q in range(WARM):
                    wq = wqp.tile([128, KT, QV], WDT, tag="wq", name=f"wq{q}b")
                    nc.sync.dma_start(wq[:], hw_[q])
                    head_group(q, 1, wq)

    return nc


def _ternary(wmat):
    """Exact {-1,0,1} ternary tensor + fp32 scale, matching the reference."""
    w = np.asarray(wmat, dtype=np.float32)
    s = np.mean(np.abs(w), dtype=np.float32)
    t = np.clip(np.rint(w / (s + np.float32(1e-8))), -1.0, 1.0).astype(np.float32)
    return t, float(s)


_NC_CACHE = {}
_LAST_RESULTS = None


def kernel(**inputs):
    global _LAST_RESULTS
    cfg = CFG_FULL
    L, H, NC, TT, V, QV, NQ = (
        cfg["L"], cfg["H"], cfg["NC"], cfg["TT"], cfg["V"], cfg["QV"], cfg["NQ"],
    )
    KT = H // 128
    TPC = TT * 128  # tokens per core
    BF = ml_dtypes.bfloat16
    F8 = ml_dtypes.float8_e4m3fn
    fp8_w = not bool(int(os.environ.get("TRIKERNEL_BF16_W", "0")))
    use_dr = fp8_w and not bool(int(os.environ.get("TRIKERNEL_NO_DR", "0")))
    WNP = F8 if fp8_w else BF

    ids = np.asarray(inputs["input_ids"]).astype(np.int64).reshape(-1)
    embed = np.asarray(inputs["embed"], dtype=np.float32)
    layer_w = np.asarray(inputs["layer_w"], dtype=np.float32)
    layer_b = np.asarray(inputs["layer_b"], dtype=np.float32)
    ln_g = np.asarray(inputs["ln_g"], dtype=np.float32)
    ln_b = np.asarray(inputs["ln_b"], dtype=np.float32)
    final_g = np.asarray(inputs["final_g"], dtype=np.float32)
    final_b = np.asarray(inputs["final_b"], dtype=np.float32)
    head_w = np.asarray(inputs["head_w"], dtype=np.float32)

    # trivial-affine specialization: the LN scale/shift and layer bias are
    # identity in this model instance; skip them on-chip when so.
    triv_ln = bool(
        np.all(ln_g == 1.0) and np.all(ln_b == 0.0) and np.all(layer_b == 0.0)
        and np.all(final_g == 1.0) and np.all(final_b == 0.0)
    )

    h0_full = embed[ids]  # [NTOK, H] fp32

    scales = []
    wT = np.empty([L, 128, KT, H], dtype=WNP)
    for l in range(L):
        t, s = _ternary(layer_w[l])
        scales.append(s)
        # [H(o), H(k)] -> transpose -> [KT,128,H] -> partition-major
        wT[l] = np.ascontiguousarray(
            t.T.reshape(KT, 128, H).transpose(1, 0, 2)
        ).astype(WNP)
    th, head_scale = _ternary(head_w)
    # head weights, vocab padded to NQ*QV, laid out so each [128, KT, QV]
    # chunk is a single contiguous 8KB-per-partition DMA: hw8[q, p, kt, v]
    thT = np.zeros((H, NQ * QV), dtype=np.float32)
    thT[:, :V] = th.T
    hw8 = np.ascontiguousarray(
        thT.reshape(KT, 128, NQ, QV).transpose(2, 1, 0, 3)
    ).astype(WNP)

    key = (tuple(sorted(cfg.items())), tuple(scales), head_scale, triv_ln,
           fp8_w, use_dr)
    if key not in _NC_CACHE:
        _NC_CACHE.clear()
        nc = build_nc(cfg, scales, head_scale, triv_ln, fp8_w, use_dr)
        # Bacc.finalize runs the TRN2 legalization passes (1-wait-per-
        # instruction event-semaphore split, matmul->ldweights wait motion,
        # register allocation). The PJRT exec path serializes nc as-is.
        nc.finalize()
        _NC_CACHE[key] = nc
    nc = _NC_CACHE[key]

    common = {
        "w": wT,
        "hw": hw8,
        "identb": np.eye(128, dtype=BF),
        "eps": np.full((128, 1), EPS, np.float32),
    }
    if not triv_ln:
        common.update(
            lng=ln_g.astype(BF),
            lnb=ln_b.astype(BF),
            lbias=layer_b.astype(BF),
            fing=final_g.astype(BF),
            finb=final_b.astype(BF),
        )
    in_maps = []
    for c in range(NC):
        h0c = np.ascontiguousarray(
            h0_full[c * TPC : (c + 1) * TPC].reshape(TT, 128, H)
        )
        # host-side pre-transpose of the layer-0 lhsT (scaled, bf16)
        h0Tc = np.ascontiguousarray(
            (h0c.reshape(TT, 128, KT, 128).transpose(0, 3, 2, 1)
             * np.float32(scales[0])).reshape(TT, 128, H)
        ).astype(BF)
        in_maps.append(dict(common, h0=h0c, h0T=h0Tc))

    trace = bool(int(os.environ.get("TRIKERNEL_TRACE", "0")))
    res = run_bass_kernel_spmd(nc, in_maps, core_ids=list(range(NC)), trace=trace)
    _LAST_RESULTS = res

    full = np.concatenate(
        [np.asarray(res.results[c]["out"]) for c in range(NC)], axis=0
    )  # [NTOK, V]
    return full.reshape(2, 1024, 32000).astype(np.float32)


# revision 26
# speedup vs baseline: 1.0022x; 1.0022x over previous
"""Trainium2 Bass kernel: 8-layer ternary (BitNet-1.58) dense transformer.

Model (per reference):
    h = embed[input_ids]                                  # (B=2, S=1024, H=2048)
    8x: y = h @ ternary(W_l)^T + b_l ; h = LN(y + h)*g+b  # H=2048
    h = LN(h)*final_g + final_b
    logits = h @ ternary(head_W)^T                        # (B, S, V=32000)

Sharding over 8 NeuronCores (fully local, no collectives):
  - Layers: data-parallel over the 2048 tokens (256 tokens/core). Each core
    streams the full ternary layer weights as exact {-1,0,+1} fp8(e4m3).
  - Head: ALSO data-parallel over tokens: each core computes its own 256
    tokens x the full 32000-entry vocab, streaming fp8 head weights
    chunk-by-chunk, overlapped with compute. No collectives at all.

Head matmul runs mixed precision: k-tiles 0..5 via fp8 DoubleRow (2 k-tiles
per instruction, activations rounded to e4m3), k-tiles 6..15 as bf16
activations x fp8 weights at full precision. The e4m3 rounding of 6/16 of
the contraction costs ~1.6e-2 relative error on the logits (vs the 2e-2
budget) and saves ~16% of the dominant matmul stream time.

When the LN affine params and biases are identity (they are for this model
instance; checked at runtime with a general fallback), the final LayerNorm
is also skipped: its input is already a LayerNorm output (per-token mean
exactly 0, variance 1-eps/var), so the final LN is an identity up to
O(eps)~2.5e-6.

HW notes (found the hard way): a ScalarE read of a full 2048B PSUM bank
hard-faults the exec unit -- all ScalarE PSUM reads here are <=1536B.
Activation transposes run as bf16 (2x faster through the PE than f32),
with a bf16 identity matrix as the moving operand.
"""

import os
import sys

import numpy as np

try:
    import concourse.bass as bass
except ImportError:  # grading container should have it on sys.path already
    sys.path.insert(0, "/opt/trn_rl_repo")
    import concourse.bass as bass

import ml_dtypes
import concourse.mybir as mybir
import concourse.tile as tile
from concourse import bacc
from concourse.bass_utils import run_bass_kernel_spmd
from contextlib import ExitStack

F32 = mybir.dt.float32
BF16 = mybir.dt.bfloat16
FP8E4 = mybir.dt.float8e4
AX = mybir.AxisListType
OP = mybir.AluOpType
AF = mybir.ActivationFunctionType
DR = mybir.MatmulPerfMode.DoubleRow
EPS = 1e-5

# Full-size problem config (B=2, S=1024 -> 2048 tokens, 256/core).
# Head: vocab padded 32000 -> 63*512; k-tiles 0..DRKT-1 run as fp8 DoubleRow.
CFG_FULL = dict(L=8, H=2048, NC=8, TT=2, V=32000, QV=512, NQ=63, CH=512, DRKT=6)


def build_nc(cfg, scales, head_scale, triv_ln, fp8_w, use_dr):
    L, H, NC, TT = cfg["L"], cfg["H"], cfg["NC"], cfg["TT"]
    V, QV, NQ, CH, DRKT = cfg["V"], cfg["QV"], cfg["NQ"], cfg["CH"], cfg["DRKT"]
    KT = H // 128
    KQ = KT // 4  # k-tiles per layer-weight quarter
    NCH = H // CH
    DRP = DRKT // 2
    if not use_dr:
        DRKT = DRP = 0
    assert H % CH == 0 and NQ * QV >= V
    WDT = FP8E4 if fp8_w else BF16

    nc = bacc.Bacc("TRN2", target_bir_lowering=False, debug=False, num_devices=NC)
    h0 = nc.declare_dram_parameter("h0", [TT, 128, H], F32, isOutput=False)
    h0T = nc.declare_dram_parameter("h0T", [TT, 128, H], BF16, isOutput=False)
    # weights pre-arranged on host: [L, 128part, KT, H] -> contiguous
    # 8KB-per-partition quarter loads (fast DMA descriptor issue)
    w_ = nc.declare_dram_parameter("w", [L, 128, KT, H], WDT, isOutput=False)
    if not triv_ln:
        lng = nc.declare_dram_parameter("lng", [L, H], BF16, isOutput=False)
        lnb = nc.declare_dram_parameter("lnb", [L, H], BF16, isOutput=False)
        lbias = nc.declare_dram_parameter("lbias", [L, H], BF16, isOutput=False)
        fing = nc.declare_dram_parameter("fing", [H], BF16, isOutput=False)
        finb = nc.declare_dram_parameter("finb", [H], BF16, isOutput=False)
    hw_ = nc.declare_dram_parameter("hw", [NQ, 128, KT, QV], WDT, isOutput=False)
    identb_d = nc.declare_dram_parameter("identb", [128, 128], BF16, isOutput=False)
    eps_d = nc.declare_dram_parameter("eps", [128, 1], F32, isOutput=False)
    out = nc.declare_dram_parameter("out", [TT * 128, V], F32, isOutput=True)

    with tile.TileContext(nc) as tc:
        with ExitStack() as ctx0:
            consts = ctx0.enter_context(tc.tile_pool(name="consts", bufs=1))
            state = ctx0.enter_context(tc.tile_pool(name="state", bufs=4))
            hTp = ctx0.enter_context(tc.tile_pool(name="hT", bufs=2))
            hT8p = ctx0.enter_context(tc.tile_pool(name="hT8", bufs=2))
            wqp = ctx0.enter_context(tc.tile_pool(name="wq", bufs=3))
            outp = ctx0.enter_context(tc.tile_pool(name="outstg", bufs=4))
            smp = ctx0.enter_context(tc.tile_pool(name="small", bufs=16))

            identb = consts.tile([128, 128], BF16, name="identb")
            nc.sync.dma_start(identb[:], identb_d[:])
            eps_t = consts.tile([128, 1], F32, name="epst")
            nc.sync.dma_start(eps_t[:], eps_d[:])

            h_cur = []
            hT_cur = []
            for t in range(TT):
                hTt = hTp.tile([128, H], BF16, tag="hT", name=f"hT_p{t}")
                hT_cur.append(hTt)
                st = state.tile([128, H], F32, name=f"hinit{t}", tag="state")
                h_cur.append(st)
            hT8_cur = [None] * TT

            with ExitStack() as ctxA:
                zpool = ctxA.enter_context(tc.tile_pool(name="z", bufs=2))
                wp = ctxA.enter_context(tc.tile_pool(name="w", bufs=12))
                sqp = ctxA.enter_context(tc.tile_pool(name="sq", bufs=2))
                hSp = ctxA.enter_context(tc.tile_pool(name="hS", bufs=2))
                gbp = None
                if not triv_ln:
                    gbp = ctxA.enter_context(tc.tile_pool(name="gb", bufs=2))
                psT = ctxA.enter_context(
                    tc.tile_pool(name="psT", bufs=2, space="PSUM")
                )
                psY = ctxA.enter_context(
                    tc.tile_pool(name="psY", bufs=NCH + 2, space="PSUM")
                )

                def load_wq(l, nsplit=8):
                    w_q = []
                    kq = KT // nsplit
                    for hf in range(nsplit):
                        wt = wp.tile([128, kq, H], WDT, tag="w", name=f"w{l}_{hf}")
                        nc.sync.dma_start(
                            wt[:], w_[l, :, hf * kq : (hf + 1) * kq, :]
                        )
                        w_q.append(wt)
                    return w_q, kq

                # first lhsT tile, then layer-0 weights in small slices so
                # the very first matmuls unblock with minimal DMA in flight
                nc.sync.dma_start(hT_cur[0][:], h0T[0])
                w_pre = load_wq(0, nsplit=8)
                nc.sync.dma_start(hT_cur[1][:], h0T[1])
                for t in range(TT):
                    nc.sync.dma_start(h_cur[t][:], h0[t])

                def transpose_bf(src_bf, name):
                    """bf16 pre-scaled [128tok, H] -> hT bf16 [128feat-in-
                    block, (kt,128tok)]; 1-bank bf16 psum halves, ScalarE
                    psum reads kept at 1024B (full-bank reads fault)."""
                    dst = hTp.tile([128, H], BF16, tag="hT", name=f"hT{name}")
                    for hf in range(2):
                        pT = psT.tile(
                            [128, H // 2], BF16, tag="psTb", name=f"pT{name}_{hf}"
                        )
                        for k in range(KT // 2):
                            kt = hf * (KT // 2) + k
                            nc.tensor.transpose(
                                pT[:, k * 128 : (k + 1) * 128],
                                src_bf[:, kt * 128 : (kt + 1) * 128],
                                identb[:],
                            )
                        lo = hf * (H // 2)
                        nc.scalar.copy(dst[:, lo : lo + 512], pT[:, :512])
                        nc.scalar.copy(dst[:, lo + 512 : lo + 1024], pT[:, 512:])
                    return dst

                def transpose_head(src_scaled_bf, src_unscaled_bf, name):
                    """Final (head) lhsT: kt 0..DRKT-1 transposed from the
                    UNSCALED bf16 activations -> e4m3 (DoubleRow operand);
                    kt DRKT..15 from the head_scale-folded bf16 ones."""
                    dst = hTp.tile([128, H], BF16, tag="hT", name=f"hT{name}")
                    dst8 = None
                    if DRKT:
                        dst8 = hT8p.tile(
                            [128, DRKT, 128], FP8E4, tag="hT8", name=f"hT8{name}"
                        )
                        pX = psT.tile(
                            [128, H // 2], BF16, tag="psTb", name=f"pX{name}"
                        )
                        for kt in range(DRKT):
                            nc.tensor.transpose(
                                pX[:, kt * 128 : (kt + 1) * 128],
                                src_unscaled_bf[:, kt * 128 : (kt + 1) * 128],
                                identb[:],
                            )
                        nc.scalar.copy(
                            dst8[:].rearrange("p a b -> p (a b)"),
                            pX[:, : DRKT * 128],
                        )
                    rem = list(range(DRKT, KT))
                    for c0 in range(0, len(rem), KT // 2):
                        grp = rem[c0 : c0 + KT // 2]
                        pT = psT.tile(
                            [128, H // 2], BF16, tag="psTb", name=f"pY{name}_{c0}"
                        )
                        for j, kt in enumerate(grp):
                            nc.tensor.transpose(
                                pT[:, j * 128 : (j + 1) * 128],
                                src_scaled_bf[:, kt * 128 : (kt + 1) * 128],
                                identb[:],
                            )
                        n = len(grp) * 128
                        lo = grp[0] * 128
                        half = min(512, n)
                        nc.scalar.copy(dst[:, lo : lo + half], pT[:, :half])
                        if half < n:
                            nc.scalar.copy(
                                dst[:, lo + half : lo + n], pT[:, half:n]
                            )
                    return dst, dst8

                def ln_core(z, sums, name):
                    """Shared LN statistics tail: returns (negmean, rstd)."""
                    S = smp.tile([128, 1], F32, tag="s0", name=f"S{name}")
                    SS = smp.tile([128, 1], F32, tag="s1", name=f"SS{name}")
                    nc.vector.tensor_reduce(
                        S[:], sums[:, 0:1], axis=AX.X, op=OP.add
                    )
                    nc.vector.tensor_reduce(
                        SS[:], sums[:, NCH : 2 * NCH], axis=AX.X, op=OP.add
                    )
                    negmean = smp.tile([128, 1], F32, tag="s2", name=f"nm{name}")
                    nc.vector.tensor_scalar_mul(negmean[:], S[:], -1.0 / H)
                    msq = smp.tile([128, 1], F32, tag="s3", name=f"msq{name}")
                    nc.vector.tensor_scalar_mul(msq[:], SS[:], 1.0 / H)
                    var = smp.tile([128, 1], F32, tag="s4", name=f"var{name}")
                    nc.vector.tensor_tensor(var[:], negmean[:], negmean[:], OP.mult)
                    nc.vector.tensor_tensor(var[:], msq[:], var[:], OP.subtract)
                    std = smp.tile([128, 1], F32, tag="s5", name=f"std{name}")
                    nc.scalar.activation(std[:], var[:], AF.Sqrt, bias=eps_t[:])
                    rstd = smp.tile([128, 1], F32, tag="s6", name=f"rstd{name}")
                    nc.vector.reciprocal(rstd[:], std[:])
                    return negmean, rstd

                fg = fb = None
                if not triv_ln:
                    fg = gbp.tile([128, H], BF16, tag="g", name="gfin")
                    nc.sync.dma_start(fg[:], fing[None, :].to_broadcast((128, H)))
                    fb = gbp.tile([128, H], BF16, tag="b", name="bfin")
                    nc.sync.dma_start(fb[:], finb[None, :].to_broadcast((128, H)))

                for l in range(L):
                    w_q, kq = w_pre if l == 0 else load_wq(l)
                    g_t = b_t = bias_t = None
                    if not triv_ln:
                        g_t = gbp.tile([128, H], BF16, tag="g", name=f"g{l}")
                        nc.sync.dma_start(
                            g_t[:], lng[l][None, :].to_broadcast((128, H))
                        )
                        b_t = gbp.tile([128, H], BF16, tag="b", name=f"b{l}")
                        nc.sync.dma_start(
                            b_t[:], lnb[l][None, :].to_broadcast((128, H))
                        )
                        bias_t = gbp.tile(
                            [128, H], BF16, tag="bias", name=f"bias{l}"
                        )
                        nc.sync.dma_start(
                            bias_t[:], lbias[l][None, :].to_broadcast((128, H))
                        )

                    last = l == L - 1
                    next_scale = scales[l + 1] if not last else head_scale
                    for t in range(TT):
                        hTt = hT_cur[t]
                        ps = [
                            psY.tile([128, CH], F32, tag="psY", name=f"ps{l}_{t}_{i}")
                            for i in range(NCH)
                        ]
                        for half in range(2):
                            for kt in range(KT):
                                wt = w_q[kt // kq]
                                for i in (2 * half, 2 * half + 1):
                                    nc.tensor.matmul(
                                        ps[i][:],
                                        lhsT=hTt[:, kt * 128 : (kt + 1) * 128],
                                        rhs=wt[:, kt % kq, i * CH : (i + 1) * CH],
                                        start=(kt == 0),
                                        stop=(kt == KT - 1),
                                    )
                        z = zpool.tile([128, H], F32, tag="z", name=f"z{l}_{t}")
                        sums = smp.tile(
                            [128, 2 * NCH], F32, tag="sums", name=f"sm{l}_{t}"
                        )
                        resid = h_cur[t]
                        if not triv_ln:
                            hb = zpool.tile(
                                [128, H], F32, tag="hb", name=f"hb{l}_{t}"
                            )
                            nc.vector.tensor_tensor(
                                hb[:], h_cur[t][:], bias_t[:], OP.add
                            )
                            resid = hb
                        for i in range(NCH):
                            nc.vector.tensor_add(
                                z[:, i * CH : (i + 1) * CH],
                                ps[i][:],
                                resid[:, i * CH : (i + 1) * CH],
                            )
                        nc.vector.tensor_reduce(
                            sums[:, 0:1], z[:], axis=AX.X, op=OP.add
                        )
                        for i in range(NCH):
                            sq = sqp.tile(
                                [128, CH], BF16, tag="sq", name=f"sq{l}_{t}_{i}"
                            )
                            nc.scalar.activation(
                                sq[:],
                                z[:, i * CH : (i + 1) * CH],
                                AF.Square,
                                accum_out=sums[:, NCH + i : NCH + i + 1],
                            )
                        negmean, rstd = ln_core(z, sums, f"{l}_{t}")
                        rstd_s = smp.tile(
                            [128, 1], F32, tag="s7", name=f"rs{l}_{t}"
                        )
                        nc.vector.tensor_scalar_mul(
                            rstd_s[:], rstd[:], float(next_scale)
                        )

                        if triv_ln and last:
                            # hn is mean-0/var-1 by construction, so the
                            # final identity-affine LN is a no-op to O(eps):
                            # emit one UNSCALED bf16 activation feeding both
                            # head operand sets -- no f32 state, no stats.
                            hnS = hSp.tile(
                                [128, H], BF16, tag="hS", name=f"hS{l}_{t}"
                            )
                            nc.vector.tensor_scalar(
                                hnS[:], z[:], negmean[:], rstd[:],
                                OP.add, OP.mult,
                            )
                            hT_cur[t], hT8_cur[t] = transpose_head(
                                hnS, hnS, f"fin{t}"
                            )
                            continue

                        hn = state.tile([128, H], F32, tag="state", name=f"h{l}_{t}")
                        nc.vector.tensor_scalar(
                            hn[:], z[:], negmean[:], rstd[:], OP.add, OP.mult
                        )
                        if not triv_ln:
                            nc.vector.tensor_tensor(hn[:], hn[:], g_t[:], OP.mult)
                            nc.vector.tensor_tensor(hn[:], hn[:], b_t[:], OP.add)
                        h_cur[t] = hn
                        if not last:
                            hnS = hSp.tile(
                                [128, H], BF16, tag="hS", name=f"hS{l}_{t}"
                            )
                            if triv_ln:
                                nc.vector.tensor_scalar(
                                    hnS[:], z[:], negmean[:], rstd_s[:],
                                    OP.add, OP.mult,
                                )
                            else:
                                nc.vector.tensor_scalar_mul(
                                    hnS[:], hn[:], float(next_scale)
                                )
                            hT_cur[t] = transpose_bf(hnS, f"{l}_{t}")

                if not triv_ln:
                    # general path: true final LN, f32 transposes
                    for t in range(TT):
                        h8 = h_cur[t]
                        sums = smp.tile(
                            [128, 2 * NCH], F32, tag="sums", name=f"smf{t}"
                        )
                        nc.vector.tensor_reduce(
                            sums[:, 0:1], h8[:], axis=AX.X, op=OP.add
                        )
                        for i in range(NCH):
                            sq = sqp.tile(
                                [128, CH], BF16, tag="sq", name=f"sqf{t}_{i}"
                            )
                            nc.scalar.activation(
                                sq[:],
                                h8[:, i * CH : (i + 1) * CH],
                                AF.Square,
                                accum_out=sums[:, NCH + i : NCH + i + 1],
                            )
                        negmean, rstd = ln_core(h8, sums, f"fin{t}")
                        hfin = state.tile(
                            [128, H], F32, tag="state", name=f"hf{t}"
                        )
                        nc.vector.tensor_scalar(
                            hfin[:], h8[:], negmean[:], rstd[:], OP.add, OP.mult
                        )
                        nc.vector.tensor_tensor(hfin[:], hfin[:], fg[:], OP.mult)
                        nc.vector.tensor_tensor(hfin[:], hfin[:], fb[:], OP.add)
                        hnS = hSp.tile([128, H], BF16, tag="hS", name=f"hSf{t}")
                        nc.vector.tensor_scalar_mul(hnS[:], hfin[:], 1.0)
                        hT_cur[t], hT8_cur[t] = transpose_head(
                            hnS, hnS, f"fin{t}"
                        )

            # ---- head: own 256 tokens x full vocab, streamed fp8 weights ----
            with ExitStack() as ctxB:
                psH = ctxB.enter_context(
                    tc.tile_pool(name="psH", bufs=6, space="PSUM")
                )

                def head_group(q, t, wq):
                    # single accumulation group: DR (e4m3) k-tiles then bf16
                    # k-tiles into one bank; both lhsT operands are UNSCALED,
                    # the ternary head scale is applied once at drain.
                    cols = min(QV, V - q * QV)
                    pb = psH.tile([128, QV], F32, tag="psH", name=f"pb{q}_{t}")
                    for j in range(DRP):
                        nc.tensor.matmul(
                            pb[:],
                            lhsT=hT8_cur[t][:, 2 * j : 2 * j + 2, :],
                            rhs=wq[:, 2 * j : 2 * j + 2, :],
                            start=(j == 0),
                            stop=False,
                            perf_mode=DR,
                            skip_group_check=True,
                        )
                    for kt in range(DRKT, KT):
                        nc.tensor.matmul(
                            pb[:],
                            lhsT=hT_cur[t][:, kt * 128 : (kt + 1) * 128],
                            rhs=wq[:, kt, :],
                            start=(DRP == 0 and kt == DRKT),
                            stop=(kt == KT - 1),
                            skip_group_check=True,
                        )
                    # ScalarE PSUM reads must stay under one full 2048B bank
                    hqv = QV // 2
                    o_t = outp.tile([128, QV], F32, tag="ostg", name=f"o{q}_{t}")
                    nc.scalar.activation(
                        o_t[:, :hqv], pb[:, :hqv], AF.Copy,
                        scale=float(head_scale),
                    )
                    nc.scalar.activation(
                        o_t[:, hqv:], pb[:, hqv:], AF.Copy,
                        scale=float(head_scale),
                    )
                    nc.sync.dma_start(
                        out[t * 128 : (t + 1) * 128, q * QV : q * QV + cols],
                        o_t[:, :cols],
                    )

                # first chunks on tile 0 only while tile 1's final
                # transposes land; their tile-1 groups run at the end
                WARM = 2
                wqb = []
                for q in range(NQ):
                    wq = wqp.tile([128, KT, QV], WDT, tag="wq", name=f"wq{q}")
                    nc.sync.dma_start(wq[:], hw_[q])
                    for t in [0] if q < WARM else range(TT):
                        head_group(q, t, wq)
                    if q == NQ - 8:
                        # prefetch the revisit chunks (dedicated tag so the
                        # main ring keeps cycling) well before the tail
                        for r in range(WARM):
                            wt = wqp.tile(
                                [128, KT, QV], WDT, tag="wqb", name=f"wq{r}b"
                            )
                            nc.sync.dma_start(wt[:], hw_[r])
                            wqb.append(wt)
                for q in range(WARM):
                    head_group(q, 1, wqb[q])

    return nc


def _ternary(wmat):
    """Exact {-1,0,1} ternary tensor + fp32 scale, matching the reference."""
    w = np.asarray(wmat, dtype=np.float32)
    s = np.mean(np.abs(w), dtype=np.float32)
    t = np.clip(np.rint(w / (s + np.float32(1e-8))), -1.0, 1.0).astype(np.float32)
    return t, float(s)


_NC_CACHE = {}
_LAST_RESULTS = None


def kernel(**inputs):
    global _LAST_RESULTS
    cfg = CFG_FULL
    L, H, NC, TT, V, QV, NQ = (
        cfg["L"], cfg["H"], cfg["NC"], cfg["TT"], cfg["V"], cfg["QV"], cfg["NQ"],
    )
    KT = H // 128
    TPC = TT * 128  # tokens per core
    BF = ml_dtypes.bfloat16
    F8 = ml_dtypes.float8_e4m3fn
    fp8_w = not bool(int(os.environ.get("TRIKERNEL_BF16_W", "0")))
    use_dr = fp8_w and not bool(int(os.environ.get("TRIKERNEL_NO_DR", "0")))
    WNP = F8 if fp8_w else BF

    ids = np.asarray(inputs["input_ids"]).astype(np.int64).reshape(-1)
    embed = np.asarray(inputs["embed"], dtype=np.float32)
    layer_w = np.asarray(inputs["layer_w"], dtype=np.float32)
    layer_b = np.asarray(inputs["layer_b"], dtype=np.float32)
    ln_g = np.asarray(inputs["ln_g"], dtype=np.float32)
    ln_b = np.asarray(inputs["ln_b"], dtype=np.float32)
    final_g = np.asarray(inputs["final_g"], dtype=np.float32)
    final_b = np.asarray(inputs["final_b"], dtype=np.float32)
    head_w = np.asarray(inputs["head_w"], dtype=np.float32)

    # trivial-affine specialization: the LN scale/shift and layer bias are
    # identity in this model instance; skip them on-chip when so.
    triv_ln = bool(
        np.all(ln_g == 1.0) and np.all(ln_b == 0.0) and np.all(layer_b == 0.0)
        and np.all(final_g == 1.0) and np.all(final_b == 0.0)
    )

    h0_full = embed[ids]  # [NTOK, H] fp32

    scales = []
    wT = np.empty([L, 128, KT, H], dtype=WNP)
    for l in range(L):
        t, s = _ternary(layer_w[l])
        scales.append(s)
        # [H(o), H(k)] -> transpose -> [KT,128,H] -> partition-major
        wT[l] = np.ascontiguousarray(
            t.T.reshape(KT, 128, H).transpose(1, 0, 2)
        ).astype(WNP)
    th, head_scale = _ternary(head_w)
    # head weights, vocab padded to NQ*QV, laid out so each [128, KT, QV]
    # chunk is a single contiguous 8KB-per-partition DMA: hw8[q, p, kt, v]
    thT = np.zeros((H, NQ * QV), dtype=np.float32)
    thT[:, :V] = th.T
    hw8 = np.ascontiguousarray(
        thT.reshape(KT, 128, NQ, QV).transpose(2, 1, 0, 3)
    ).astype(WNP)

    key = (tuple(sorted(cfg.items())), tuple(scales), head_scale, triv_ln,
           fp8_w, use_dr)
    if key not in _NC_CACHE:
        _NC_CACHE.clear()
        nc = build_nc(cfg, scales, head_scale, triv_ln, fp8_w, use_dr)
        # Bacc.finalize runs the TRN2 legalization passes (1-wait-per-
        # instruction event-semaphore split, matmul->ldweights wait motion,
        # register allocation). The PJRT exec path serializes nc as-is.
        nc.finalize()
        _NC_CACHE[key] = nc
    nc = _NC_CACHE[key]

    common = {
        "w": wT,
        "hw": hw8,
        "identb": np.eye(128, dtype=BF),
        "eps": np.full((128, 1), EPS, np.float32),
    }
    if not triv_ln:
        common.update(
            lng=ln_g.astype(BF),
            lnb=ln_b.astype(BF),
            lbias=layer_b.astype(BF),
            fing=final_g.astype(BF),
            finb=final_b.astype(BF),
        )
    in_maps = []
    for c in range(NC):
        h0c = np.ascontiguousarray(
            h0_full[c * TPC : (c + 1) * TPC].reshape(TT, 128, H)
        )
        # host-side pre-transpose of the layer-0 lhsT (scaled, bf16)
        h0Tc = np.ascontiguousarray(
            (h0c.reshape(TT, 128, KT, 128).transpose(0, 3, 2, 1)
             * np.float32(scales[0])).reshape(TT, 128, H)
        ).astype(BF)
        in_maps.append(dict(common, h0=h0c, h0T=h0Tc))

    trace = bool(int(os.environ.get("TRIKERNEL_TRACE", "0")))
    res = run_bass_kernel_spmd(nc, in_maps, core_ids=list(range(NC)), trace=trace)
    _LAST_RESULTS = res

    full = np.concatenate(
        [np.asarray(res.results[c]["out"]) for c in range(NC)], axis=0
    )  # [NTOK, V]
    return full.reshape(2, 1024, 32000).astype(np.float32)
